# revision 15
# baseline (speedup 1.0000x reference)
"""Trainium2 Bass kernel for nn_PredictionNet — data-parallel over batch.

8-way batch sharding (32 rows/core), all expert weights replicated per core
(17.7MB fp16, SBUF-resident). No cross-core communication — NRT collectives
have a ~66us pipeline floor in this environment, far above their usefulness.

The kernel is weight-DMA-bound (~17.7MB at ~330GB/s aggregate over the 3
trigger queues when transfers are chunked ~0.8MB and round-robined). Compute
rides under the DMA: per expert, 13/5/5 matmuls (fp16, fp32 psum) with the
bias+z contribution folded into host-prescaled `zce` stationaries (zc*coef_e,
so the z-part lands pre-blended via accumulating matmuls); the per-sample
blend is an eager scalar/vector chain per expert psum; ELU on vector+Act;
PE transposes re-K-major the hidden state between layers.
"""

import sys

sys.path.insert(0, "/opt/trn_rl_repo")

import numpy as np

import concourse.bass as bass
import concourse.mybir as mybir
import concourse.tile as tile
from concourse.bass_utils import run_bass_kernel_spmd

B, E = 256, 6
IN, HID, OUT, ZD = 1664, 512, 618, 32
N_CORES = 8
CORE_IDS = list(range(N_CORES))
BC = B // N_CORES         # 32 batch rows per core
K1 = IN // 128            # 13 k-chunks, layer 1
KH = HID // 128           # 4 k-chunks for the hidden part of layers 2/3
OUTP = 640                # layer-3 output padded 618 -> 640
OH3 = OUTP // 2           # 320: layer-3 psum half width
ZR = 1 + ZD               # 33: ones row + z rows
FP32 = mybir.dt.float32
FP16 = mybir.dt.float16
ALU = mybir.AluOpType
ACT = mybir.ActivationFunctionType


def _split_waits(nc, max_waits=1):
    """neuronxcc walrus accepts only ONE sync-wait per instruction: hoist
    extras onto same-engine NoOps placed before the offending instruction."""
    n = 0
    for fn in nc.m.functions:
        for blk in fn.blocks:
            insts = blk.instructions
            if not any(
                i.sync_info is not None and len(i.sync_info.on_wait) > max_waits
                for i in insts
            ):
                continue
            out = []
            for inst in insts:
                si = inst.sync_info
                if si is not None and len(si.on_wait) > max_waits:
                    for w in si.on_wait[:-max_waits]:
                        n += 1
                        nop = mybir.InstNoOp(name=f"I-wfix{n}", ins=[], outs=[])
                        nop.engine = inst.engine
                        nop.sync_info = mybir.SyncInfo(on_wait=[w], on_update=[])
                        try:
                            nc.register_instruction(nop, overwrite=True)
                        except Exception:
                            pass
                        out.append(nop)
                    inst.sync_info = mybir.SyncInfo(
                        on_wait=list(si.on_wait[-max_waits:]),
                        on_update=list(si.on_update),
                    )
                out.append(inst)
            blk.instructions = out
    return n


def _trim_tail(nc):
    """Drop the second all-engine barrier round + sem-clear at the kernel
    tail: the first drain+barrier already guarantees completion, and the
    preamble re-initializes semaphores on any re-execution."""
    blk = nc.m.functions[0].blocks[-1]
    insts = blk.instructions
    cut = None
    for idx in range(len(insts) - 1, -1, -1):
        if type(insts[idx]).__name__ == "InstISA":
            cut = idx
            break
    if cut is not None:
        blk.instructions = insts[:cut]


def build_nc():
    nc = bass.Bass()

    hc_d = nc.dram_tensor("hc", [128, K1, BC], FP16, kind="ExternalInput")
    ones_d = nc.dram_tensor("ones", [1, BC], FP16, kind="ExternalInput")
    zce_d = nc.dram_tensor("zce", [ZR, E, BC], FP16, kind="ExternalInput")
    coefc_d = nc.dram_tensor("coefc", [BC, E], FP32, kind="ExternalInput")
    idn_d = nc.dram_tensor("idn", [BC, BC], FP16, kind="ExternalInput")
    b1_d = nc.dram_tensor("b1cat", [1, E, HID], FP16, kind="ExternalInput")
    w1_d = nc.dram_tensor("w1cat", [128, K1, E, HID], FP16, kind="ExternalInput")
    w2z_d = nc.dram_tensor("w2zcat", [ZR, E, HID], FP16, kind="ExternalInput")
    w2_d = nc.dram_tensor("w2cat", [128, KH, E, HID], FP16, kind="ExternalInput")
    w3z_d = nc.dram_tensor("w3zcat", [ZR, E, OUTP], FP16, kind="ExternalInput")
    w3_d = nc.dram_tensor("w3cat", [128, KH, E, OUTP], FP16, kind="ExternalInput")
    out_d = nc.dram_tensor("outc", [BC, OUTP], FP32, kind="ExternalOutput")

    with tile.TileContext(nc) as tc:
        with (
            tc.tile_pool(name="const", bufs=1) as cp,
            tc.tile_pool(name="work", bufs=1) as wp,
            tc.tile_pool(name="psum", bufs=3, space="PSUM") as pp,
            tc.tile_pool(name="psumt", bufs=2, space="PSUM") as pt,
        ):
            # ---------------- DMAs ------------------------------------------
            # Distinct in-flight transfers parallelize across SDMA engines, so
            # chunk ~0.8MB and round-robin the 3 queues in consumption order.
            queues = [nc.sync, nc.scalar, nc.gpsimd]
            qi = 0

            def q():
                nonlocal qi
                e = queues[qi % 3]
                qi += 1
                return e

            ones_t = cp.tile([1, BC], FP16)
            nc.gpsimd.dma_start(out=ones_t[:], in_=ones_d[:])
            b1 = cp.tile([1, E, HID], FP16)
            nc.gpsimd.dma_start(out=b1[:], in_=b1_d[:])
            zce = cp.tile([ZR, E, BC], FP16)
            nc.scalar.dma_start(out=zce[:], in_=zce_d[:])
            coefc = cp.tile([BC, E], FP32)
            nc.scalar.dma_start(out=coefc[:], in_=coefc_d[:])
            idn = cp.tile([BC, BC], FP16)
            nc.sync.dma_start(out=idn[:], in_=idn_d[:])
            hc = cp.tile([128, K1, BC], FP16)
            nc.sync.dma_start(out=hc[:], in_=hc_d[:])
            w1 = cp.tile([128, K1, E, HID], FP16)
            for k in range(K1):
                q().dma_start(out=w1[:, k], in_=w1_d[:, k])
            w2z = cp.tile([ZR, E, HID], FP16)
            q().dma_start(out=w2z[:], in_=w2z_d[:])
            w2 = cp.tile([128, KH, E, HID], FP16)
            for k in range(KH):
                q().dma_start(out=w2[:, k], in_=w2_d[:, k])
            w3z = cp.tile([ZR, E, OUTP], FP16)
            q().dma_start(out=w3z[:], in_=w3z_d[:])
            w3 = cp.tile([128, KH, E, OUTP], FP16)
            for k in range(KH):
                q().dma_start(out=w3[:, k, :, 0:OH3], in_=w3_d[:, k, :, 0:OH3])
                q().dma_start(out=w3[:, k, :, OH3:], in_=w3_d[:, k, :, OH3:])

            # preload the Exp activation table off the critical path
            scratch = wp.tile([1, BC], FP32, tag="scratch")
            nc.scalar.activation(scratch[:], ones_t[:], ACT.Exp)

            def blend_step(acc_prev, ps, e, tag, half=None):
                """eager per-expert blend: acc = ps*coef_e + acc_prev.
                Alternates vector stt with scalar-engine scaled copies to
                split load; scalar path needs a vector add to merge."""
                w = ps.shape[-1]
                acc = wp.tile([BC, w], FP32, name=f"{tag}_acc{e}",
                              tag=f"{tag}_acc", bufs=2)
                if acc_prev is None:
                    nc.scalar.mul(acc[:], ps[:], coefc[:, e : e + 1])
                else:
                    nc.vector.scalar_tensor_tensor(
                        acc[:], ps[:], coefc[:, e : e + 1], acc_prev[:],
                        ALU.mult, ALU.add,
                    )
                return acc

            def elu_f16(acc, tag, w=HID):
                tneg = wp.tile([BC, w], FP32, tag=f"{tag}_neg")
                nc.vector.tensor_scalar_min(tneg[:], acc[:], 0.0)
                texp = wp.tile([BC, w], FP16, tag=f"{tag}_exp")
                nc.scalar.activation(texp[:], tneg[:], ACT.Exp)
                trel = wp.tile([BC, w], FP16, tag=f"{tag}_rel")
                nc.scalar.activation(trel[:], acc[:], ACT.Relu)
                res = wp.tile([BC, w], FP16, tag=f"{tag}_res")
                nc.vector.scalar_tensor_tensor(
                    res[:], texp[:], -1.0, trel[:], ALU.add, ALU.add
                )
                return res

            def transpose_kmajor(h, tag):
                """h [32, 512] fp16 -> K-major fp16 [128, 4, 32]."""
                ht = wp.tile([128, KH, BC], FP16, name=f"{tag}_ht", tag=f"{tag}_ht")
                for j in range(KH):
                    ps = pt.tile([128, BC], FP16, name=f"{tag}_tp{j}", tag="tpose")
                    nc.tensor.transpose(
                        ps[:], h[:, j * 128 : (j + 1) * 128], idn[:]
                    )
                    eng = nc.vector if j % 2 == 0 else nc.scalar
                    if j % 2 == 0:
                        nc.vector.tensor_copy(ht[:, j, :], ps[:])
                    else:
                        nc.scalar.copy(ht[:, j, :], ps[:])
                return ht

            # ================= Layer 1 =================
            acc = None
            for e in range(E):
                ps = pp.tile([BC, HID], FP32, name=f"l1ps{e}", tag="ps")
                nc.tensor.matmul(
                    ps[:], ones_t[:], b1[:, e, :], start=True, stop=False
                )
                for k in range(K1):
                    nc.tensor.matmul(
                        ps[:], hc[:, k, :], w1[:, k, e, :],
                        start=False, stop=(k == K1 - 1),
                    )
                acc = blend_step(acc, ps, e, "l1")
            h1 = elu_f16(acc, "l1")
            h1t = transpose_kmajor(h1, "l1")

            # ================= Layer 2 =================
            # z+bias pre-blended via zce (zc*coef_e) accumulating matmuls
            zps2 = pp.tile([BC, HID], FP32, name="zps2", tag="zps")
            for e in range(E):
                nc.tensor.matmul(
                    zps2[:], zce[:, e, :], w2z[:, e, :],
                    start=(e == 0), stop=(e == E - 1),
                )
            acc = None
            for e in range(E):
                ps = pp.tile([BC, HID], FP32, name=f"l2ps{e}", tag="ps")
                for k in range(KH):
                    nc.tensor.matmul(
                        ps[:], h1t[:, k, :], w2[:, k, e, :],
                        start=(k == 0), stop=(k == KH - 1),
                    )
                acc = blend_step(acc, ps, e, "l2")
            pre2 = wp.tile([BC, HID], FP32, tag="pre2")
            nc.vector.scalar_tensor_tensor(
                pre2[:], acc[:], 1.0, zps2[:], ALU.mult, ALU.add
            )
            h2 = elu_f16(pre2, "l2")
            h2t = transpose_kmajor(h2, "l2")

            # ================= Layer 3 ================= (two 320 halves)
            res3 = wp.tile([BC, OUTP], FP32, tag="res3")
            for half in range(2):
                sl = slice(half * OH3, (half + 1) * OH3)
                zps3 = pp.tile([BC, OH3], FP32, name=f"zps3_{half}", tag="zps")
                for e in range(E):
                    nc.tensor.matmul(
                        zps3[:], zce[:, e, :], w3z[:, e, sl],
                        start=(e == 0), stop=(e == E - 1),
                    )
                zsb3 = wp.tile([BC, OH3], FP32, tag=f"zsb3_{half}")
                nc.scalar.copy(zsb3[:], zps3[:])
                acc = None
                for e in range(E):
                    ps = pp.tile([BC, OH3], FP32, name=f"l3ps{half}_{e}", tag="ps")
                    for k in range(KH):
                        nc.tensor.matmul(
                            ps[:], h2t[:, k, :], w3[:, k, e, sl],
                            start=(k == 0), stop=(k == KH - 1),
                        )
                    if e == 0:
                        acc = wp.tile([BC, OH3], FP32, name=f"l3a{half}",
                                      tag="l3_acc", bufs=2)
                        nc.vector.scalar_tensor_tensor(
                            acc[:], ps[:], coefc[:, 0:1], zsb3[:],
                            ALU.mult, ALU.add,
                        )
                    else:
                        acc = blend_step(acc, ps, e, f"l3h{half}")
                nc.vector.tensor_copy(res3[:, sl], acc[:])
            nc.sync.dma_start(out=out_d[:], in_=res3[:])

    _split_waits(nc)
    _trim_tail(nc)
    return nc


_NC_CACHE = None


def _get_nc():
    global _NC_CACHE
    if _NC_CACHE is None:
        _NC_CACHE = build_nc()
    return _NC_CACHE


def make_in_maps(p_prev, blending_coef, z, w_l1, b_l1, w_l2, b_l2, w_l3, b_l3):
    f, h = np.float32, np.float16
    h0 = np.concatenate([z, p_prev], axis=1).astype(f)            # [B, IN]
    coef = np.asarray(blending_coef).astype(f)                    # [B, E]

    w1cat = np.ascontiguousarray(                                 # [128,13,E,512]
        w_l1.astype(h).reshape(E, K1, 128, HID).transpose(2, 1, 0, 3)
    )
    b1cat = np.ascontiguousarray(b_l1.astype(h)[None])            # [1, E, 512]
    w2zcat = np.ascontiguousarray(                                # [33, E, 512]
        np.concatenate(
            [b_l2.astype(f)[:, None, :], w_l2[:, :ZD, :].astype(f)], axis=1
        ).transpose(1, 0, 2).astype(h)
    )
    w2cat = np.ascontiguousarray(                                 # [128,4,E,512]
        w_l2[:, ZD:, :].astype(h).reshape(E, KH, 128, HID).transpose(2, 1, 0, 3)
    )
    w3pad = np.zeros((E, ZD + HID, OUTP), f)
    w3pad[:, :, :OUT] = w_l3
    b3pad = np.zeros((E, OUTP), f)
    b3pad[:, :OUT] = b_l3
    w3zcat = np.ascontiguousarray(                                # [33, E, 640]
        np.concatenate([b3pad[:, None, :], w3pad[:, :ZD, :]], axis=1)
        .transpose(1, 0, 2).astype(h)
    )
    w3cat = np.ascontiguousarray(                                 # [128,4,E,640]
        w3pad[:, ZD:, :].astype(h).reshape(E, KH, 128, OUTP).transpose(2, 1, 0, 3)
    )
    ones = np.ones((1, BC), h)
    idn = np.eye(BC, dtype=h)

    in_maps = []
    for c in range(N_CORES):
        bs = slice(c * BC, (c + 1) * BC)
        hc = np.ascontiguousarray(
            h0[bs].T.reshape(K1, 128, BC).transpose(1, 0, 2)
        ).astype(h)                                               # [128, 13, 32]
        zc_full = np.concatenate([np.ones((1, BC), f), z[bs].T.astype(f)], 0)
        zce = np.ascontiguousarray(                               # [33, E, 32]
            (zc_full[:, None, :] * coef[bs].T[None, :, :]).astype(h)
        )
        in_maps.append(
            {
                "hc": hc, "ones": ones, "zce": zce,
                "coefc": np.ascontiguousarray(coef[bs]), "idn": idn,
                "b1cat": b1cat, "w1cat": w1cat, "w2zcat": w2zcat,
                "w2cat": w2cat, "w3zcat": w3zcat, "w3cat": w3cat,
            }
        )
    return in_maps


def assemble_output(results):
    full = np.concatenate(
        [results[c]["outc"] for c in range(N_CORES)], axis=0
    )                                                             # [256, 640]
    return np.ascontiguousarray(full[:, :OUT]).astype(np.float32)


def kernel(p_prev, blending_coef, z, w_l1, b_l1, w_l2, b_l2, w_l3, b_l3):
    args = [
        np.asarray(a)
        for a in (p_prev, blending_coef, z, w_l1, b_l1, w_l2, b_l2, w_l3, b_l3)
    ]
    nc = _get_nc()
    in_maps = make_in_maps(*args)
    res = run_bass_kernel_spmd(nc, in_maps, CORE_IDS)
    return assemble_output(res.results)


# revision 18
# speedup vs baseline: 1.2421x; 1.2421x over previous
"""Trainium2 Bass kernel for nn_PredictionNet — data-parallel over batch.

8-way batch sharding (32 rows/core), all expert weights replicated per core
(17.7MB fp16, SBUF-resident). No cross-core communication — NRT collectives
have a ~66us pipeline floor in this environment, far above their usefulness.

The kernel is weight-DMA-bound (~17.7MB at ~330GB/s aggregate over the 3
trigger queues when transfers are chunked ~0.8MB and round-robined). Compute
rides under the DMA: per expert, 13/5/5 matmuls (fp16, fp32 psum) with the
bias+z contribution folded into host-prescaled `zce` stationaries (zc*coef_e,
so the z-part lands pre-blended via accumulating matmuls); the per-sample
blend is an eager scalar/vector chain per expert psum; ELU on vector+Act;
PE transposes re-K-major the hidden state between layers.
"""

import sys

sys.path.insert(0, "/opt/trn_rl_repo")

import numpy as np

import concourse.bass as bass
import concourse.mybir as mybir
import concourse.tile as tile
from concourse.bass_utils import run_bass_kernel_spmd

B, E = 256, 6
IN, HID, OUT, ZD = 1664, 512, 618, 32
N_CORES = 8
CORE_IDS = list(range(N_CORES))
BC = B // N_CORES         # 32 batch rows per core
K1 = IN // 128            # 13 k-chunks, layer 1
KH = HID // 128           # 4 k-chunks for the hidden part of layers 2/3
OUTP = 640                # layer-3 output padded 618 -> 640
OH3 = OUTP // 2           # 320: layer-3 psum half width
ZR = 1 + ZD               # 33: ones row + z rows
FP32 = mybir.dt.float32
FP16 = mybir.dt.float16
ALU = mybir.AluOpType
ACT = mybir.ActivationFunctionType


def _split_waits(nc, max_waits=1):
    """neuronxcc walrus accepts only ONE sync-wait per instruction: hoist
    extras onto same-engine NoOps placed before the offending instruction."""
    n = 0
    for fn in nc.m.functions:
        for blk in fn.blocks:
            insts = blk.instructions
            if not any(
                i.sync_info is not None and len(i.sync_info.on_wait) > max_waits
                for i in insts
            ):
                continue
            out = []
            for inst in insts:
                si = inst.sync_info
                if si is not None and len(si.on_wait) > max_waits:
                    for w in si.on_wait[:-max_waits]:
                        n += 1
                        nop = mybir.InstNoOp(name=f"I-wfix{n}", ins=[], outs=[])
                        nop.engine = inst.engine
                        nop.sync_info = mybir.SyncInfo(on_wait=[w], on_update=[])
                        try:
                            nc.register_instruction(nop, overwrite=True)
                        except Exception:
                            pass
                        out.append(nop)
                    inst.sync_info = mybir.SyncInfo(
                        on_wait=list(si.on_wait[-max_waits:]),
                        on_update=list(si.on_update),
                    )
                out.append(inst)
            blk.instructions = out
    return n


def _trim_tail(nc):
    """Drop the second all-engine barrier round + sem-clear at the kernel
    tail: the first drain+barrier already guarantees completion, and the
    preamble re-initializes semaphores on any re-execution."""
    blk = nc.m.functions[0].blocks[-1]
    insts = blk.instructions
    cut = None
    for idx in range(len(insts) - 1, -1, -1):
        if type(insts[idx]).__name__ == "InstISA":
            cut = idx
            break
    if cut is not None:
        blk.instructions = insts[:cut]


def build_nc():
    nc = bass.Bass()

    hc_d = nc.dram_tensor("hc", [128, K1, BC], FP16, kind="ExternalInput")
    ones_d = nc.dram_tensor("ones", [1, BC], FP16, kind="ExternalInput")
    zce_d = nc.dram_tensor("zce", [ZR, E, BC], FP16, kind="ExternalInput")
    coefc_d = nc.dram_tensor("coefc", [BC, E], FP32, kind="ExternalInput")
    idn_d = nc.dram_tensor("idn", [BC, BC], FP16, kind="ExternalInput")
    b1_d = nc.dram_tensor("b1cat", [1, E, HID], FP16, kind="ExternalInput")
    w1_d = nc.dram_tensor("w1cat", [128, K1, E, HID], FP16, kind="ExternalInput")
    w2z_d = nc.dram_tensor("w2zcat", [ZR, E, HID], FP16, kind="ExternalInput")
    w2_d = nc.dram_tensor("w2cat", [128, KH, E, HID], FP16, kind="ExternalInput")
    w3z_d = nc.dram_tensor("w3zcat", [ZR, E, OUTP], FP16, kind="ExternalInput")
    w3_d = nc.dram_tensor("w3cat", [128, KH, E, OUTP], FP16, kind="ExternalInput")
    out_d = nc.dram_tensor("outc", [BC, OUTP], FP32, kind="ExternalOutput")

    with tile.TileContext(nc) as tc:
        with (
            tc.tile_pool(name="const", bufs=1) as cp,
            tc.tile_pool(name="work", bufs=1) as wp,
            tc.tile_pool(name="psum", bufs=1, space="PSUM") as pp,
            tc.tile_pool(name="psumt", bufs=1, space="PSUM") as pt,
        ):
            # ---------------- DMAs ------------------------------------------
            # Distinct in-flight transfers parallelize across SDMA engines, so
            # chunk ~0.8MB and round-robin the 3 queues in consumption order.
            queues = [nc.sync, nc.scalar]
            qi = 0

            def q():
                nonlocal qi
                e = queues[qi % 2]
                qi += 1
                return e

            ones_t = cp.tile([1, BC], FP16)
            nc.gpsimd.dma_start(out=ones_t[:], in_=ones_d[:])
            b1 = cp.tile([1, E, HID], FP16)
            nc.gpsimd.dma_start(out=b1[:], in_=b1_d[:])
            zce = cp.tile([ZR, E, BC], FP16)
            nc.scalar.dma_start(out=zce[:], in_=zce_d[:])
            coefc = cp.tile([BC, E], FP32)
            nc.scalar.dma_start(out=coefc[:], in_=coefc_d[:])
            idn = cp.tile([BC, BC], FP16)
            nc.sync.dma_start(out=idn[:], in_=idn_d[:])
            hc = cp.tile([128, K1, BC], FP16)
            nc.sync.dma_start(out=hc[:], in_=hc_d[:])
            w1 = cp.tile([128, K1, E, HID], FP16)
            for k in range(K1):
                q().dma_start(out=w1[:, k], in_=w1_d[:, k])
            w2z = cp.tile([ZR, E, HID], FP16)
            q().dma_start(out=w2z[:], in_=w2z_d[:])
            w2 = cp.tile([128, KH, E, HID], FP16)
            for k in range(KH):
                q().dma_start(out=w2[:, k], in_=w2_d[:, k])
            w3z = cp.tile([ZR, E, OUTP], FP16)
            q().dma_start(out=w3z[:], in_=w3z_d[:])
            w3 = cp.tile([128, KH, E, OUTP], FP16)
            for k in range(KH):
                q().dma_start(out=w3[:, k, :, 0:OH3], in_=w3_d[:, k, :, 0:OH3])
                q().dma_start(out=w3[:, k, :, OH3:], in_=w3_d[:, k, :, OH3:])

            # preload the Exp activation table off the critical path
            scratch = wp.tile([1, BC], FP32, tag="scratch")
            nc.scalar.activation(scratch[:], ones_t[:], ACT.Exp)

            # PE p-state warm-up: ~4us of junk matmuls on the identity tile
            # while the first weight chunks stream in.
            warm_ps = pp.tile([BC, BC], FP32, name="warm_ps", tag="warm")
            for i in range(36):
                nc.tensor.matmul(
                    warm_ps[:], idn[:], idn[:], start=True, stop=True
                )

            def blend_step(acc_prev, ps, e, tag, half=None):
                """eager per-expert blend: acc = ps*coef_e + acc_prev.
                Alternates vector stt with scalar-engine scaled copies to
                split load; scalar path needs a vector add to merge."""
                w = ps.shape[-1]
                acc = wp.tile([BC, w], FP32, name=f"{tag}_acc{e}",
                              tag=f"{tag}_acc", bufs=2)
                if acc_prev is None:
                    nc.scalar.mul(acc[:], ps[:], coefc[:, e : e + 1])
                else:
                    nc.vector.scalar_tensor_tensor(
                        acc[:], ps[:], coefc[:, e : e + 1], acc_prev[:],
                        ALU.mult, ALU.add,
                    )
                return acc

            def elu_f16(acc, tag, w=HID):
                tneg = wp.tile([BC, w], FP32, tag=f"{tag}_neg")
                nc.vector.tensor_scalar_min(tneg[:], acc[:], 0.0)
                texp = wp.tile([BC, w], FP16, tag=f"{tag}_exp")
                nc.scalar.activation(texp[:], tneg[:], ACT.Exp)
                trel = wp.tile([BC, w], FP16, tag=f"{tag}_rel")
                nc.scalar.activation(trel[:], acc[:], ACT.Relu)
                res = wp.tile([BC, w], FP16, tag=f"{tag}_res")
                nc.vector.scalar_tensor_tensor(
                    res[:], texp[:], -1.0, trel[:], ALU.add, ALU.add
                )
                return res

            def transpose_kmajor(h, tag):
                """h [32, 512] fp16 -> K-major fp16 [128, 4, 32]."""
                ht = wp.tile([128, KH, BC], FP16, name=f"{tag}_ht", tag=f"{tag}_ht")
                for j in range(KH):
                    ps = pt.tile([128, BC], FP16, name=f"{tag}_tp{j}", tag="tpose")
                    nc.tensor.transpose(
                        ps[:], h[:, j * 128 : (j + 1) * 128], idn[:]
                    )
                    eng = nc.vector if j % 2 == 0 else nc.scalar
                    if j % 2 == 0:
                        nc.vector.tensor_copy(ht[:, j, :], ps[:])
                    else:
                        nc.scalar.copy(ht[:, j, :], ps[:])
                return ht

            # ================= Layer 1 ================= (e-outer: expert
            # psums complete staggered so blends overlap the next expert's PE)
            acc = None
            for e in range(E):
                ps = pp.tile([BC, HID], FP32, name=f"l1ps{e}", tag="ps", bufs=3)
                nc.tensor.matmul(
                    ps[:], ones_t[:], b1[:, e, :], start=True, stop=False
                )
                for k in range(K1):
                    nc.tensor.matmul(
                        ps[:], hc[:, k, :], w1[:, k, e, :],
                        start=False, stop=(k == K1 - 1),
                    )
                acc = blend_step(acc, ps, e, "l1")
            h1 = elu_f16(acc, "l1")
            h1t = transpose_kmajor(h1, "l1")

            # ================= Layer 2 =================
            # z+bias pre-blended via zce (zc*coef_e) accumulating matmuls
            zps2 = pp.tile([BC, HID], FP32, name="zps2", tag="zps")
            for e in range(E):
                nc.tensor.matmul(
                    zps2[:], zce[:, e, :], w2z[:, e, :],
                    start=(e == 0), stop=(e == E - 1),
                )
            acc = None
            for e in range(E):
                ps = pp.tile([BC, HID], FP32, name=f"l2ps{e}", tag="ps", bufs=3)
                for k in range(KH):
                    nc.tensor.matmul(
                        ps[:], h1t[:, k, :], w2[:, k, e, :],
                        start=(k == 0), stop=(k == KH - 1),
                    )
                acc = blend_step(acc, ps, e, "l2")
            pre2 = wp.tile([BC, HID], FP32, tag="pre2")
            nc.vector.scalar_tensor_tensor(
                pre2[:], acc[:], 1.0, zps2[:], ALU.mult, ALU.add
            )
            h2 = elu_f16(pre2, "l2")
            h2t = transpose_kmajor(h2, "l2")

            # ================= Layer 3 ================= (two 320 halves)
            res3 = wp.tile([BC, OUTP], FP32, tag="res3")
            for half in range(2):
                sl = slice(half * OH3, (half + 1) * OH3)
                zps3 = pp.tile([BC, OH3], FP32, name=f"zps3_{half}", tag="zps")
                for e in range(E):
                    nc.tensor.matmul(
                        zps3[:], zce[:, e, :], w3z[:, e, sl],
                        start=(e == 0), stop=(e == E - 1),
                    )
                zsb3 = wp.tile([BC, OH3], FP32, tag=f"zsb3_{half}")
                nc.scalar.copy(zsb3[:], zps3[:])
                acc = None
                for e in range(E):
                    ps = pp.tile([BC, OH3], FP32, name=f"l3ps{half}_{e}",
                                 tag="ps", bufs=3)
                    for k in range(KH):
                        nc.tensor.matmul(
                            ps[:], h2t[:, k, :], w3[:, k, e, sl],
                            start=(k == 0), stop=(k == KH - 1),
                        )
                    if e == 0:
                        acc = wp.tile([BC, OH3], FP32, name=f"l3a{half}",
                                      tag="l3_acc", bufs=2)
                        nc.vector.scalar_tensor_tensor(
                            acc[:], ps[:], coefc[:, 0:1], zsb3[:],
                            ALU.mult, ALU.add,
                        )
                    else:
                        acc = blend_step(acc, ps, e, f"l3h{half}")
                nc.vector.tensor_copy(res3[:, sl], acc[:])
            nc.sync.dma_start(out=out_d[:], in_=res3[:])

    _split_waits(nc)
    _trim_tail(nc)
    return nc


_NC_CACHE = None


def _get_nc():
    global _NC_CACHE
    if _NC_CACHE is None:
        _NC_CACHE = build_nc()
    return _NC_CACHE


def make_in_maps(p_prev, blending_coef, z, w_l1, b_l1, w_l2, b_l2, w_l3, b_l3):
    f, h = np.float32, np.float16
    h0 = np.concatenate([z, p_prev], axis=1).astype(f)            # [B, IN]
    coef = np.asarray(blending_coef).astype(f)                    # [B, E]

    w1cat = np.ascontiguousarray(                                 # [128,13,E,512]
        w_l1.astype(h).reshape(E, K1, 128, HID).transpose(2, 1, 0, 3)
    )
    b1cat = np.ascontiguousarray(b_l1.astype(h)[None])            # [1, E, 512]
    w2zcat = np.ascontiguousarray(                                # [33, E, 512]
        np.concatenate(
            [b_l2.astype(f)[:, None, :], w_l2[:, :ZD, :].astype(f)], axis=1
        ).transpose(1, 0, 2).astype(h)
    )
    w2cat = np.ascontiguousarray(                                 # [128,4,E,512]
        w_l2[:, ZD:, :].astype(h).reshape(E, KH, 128, HID).transpose(2, 1, 0, 3)
    )
    w3pad = np.zeros((E, ZD + HID, OUTP), f)
    w3pad[:, :, :OUT] = w_l3
    b3pad = np.zeros((E, OUTP), f)
    b3pad[:, :OUT] = b_l3
    w3zcat = np.ascontiguousarray(                                # [33, E, 640]
        np.concatenate([b3pad[:, None, :], w3pad[:, :ZD, :]], axis=1)
        .transpose(1, 0, 2).astype(h)
    )
    w3cat = np.ascontiguousarray(                                 # [128,4,E,640]
        w3pad[:, ZD:, :].astype(h).reshape(E, KH, 128, OUTP).transpose(2, 1, 0, 3)
    )
    ones = np.ones((1, BC), h)
    idn = np.eye(BC, dtype=h)

    in_maps = []
    for c in range(N_CORES):
        bs = slice(c * BC, (c + 1) * BC)
        hc = np.ascontiguousarray(
            h0[bs].T.reshape(K1, 128, BC).transpose(1, 0, 2)
        ).astype(h)                                               # [128, 13, 32]
        zc_full = np.concatenate([np.ones((1, BC), f), z[bs].T.astype(f)], 0)
        zce = np.ascontiguousarray(                               # [33, E, 32]
            (zc_full[:, None, :] * coef[bs].T[None, :, :]).astype(h)
        )
        in_maps.append(
            {
                "hc": hc, "ones": ones, "zce": zce,
                "coefc": np.ascontiguousarray(coef[bs]), "idn": idn,
                "b1cat": b1cat, "w1cat": w1cat, "w2zcat": w2zcat,
                "w2cat": w2cat, "w3zcat": w3zcat, "w3cat": w3cat,
            }
        )
    return in_maps


def assemble_output(results):
    full = np.concatenate(
        [results[c]["outc"] for c in range(N_CORES)], axis=0
    )                                                             # [256, 640]
    return np.ascontiguousarray(full[:, :OUT]).astype(np.float32)


def kernel(p_prev, blending_coef, z, w_l1, b_l1, w_l2, b_l2, w_l3, b_l3):
    args = [
        np.asarray(a)
        for a in (p_prev, blending_coef, z, w_l1, b_l1, w_l2, b_l2, w_l3, b_l3)
    ]
    nc = _get_nc()
    in_maps = make_in_maps(*args)
    res = run_bass_kernel_spmd(nc, in_maps, CORE_IDS)
    return assemble_output(res.results)
